# revision 22
# baseline (speedup 1.0000x reference)
"""Trainium2 Bass kernel for nn_Circuit_19275813225041 (v3).

24-qubit state-vector simulation: one layer of single-qubit gates on every
qubit, then a ladder of two-qubit gates on neighboring pairs (q, q+1),
q = 0..22, on a 2^24 complex state stored as (2, 2^24) float32 (re, im).

v3 strategy (8 NeuronCores): fused matmul-transposes, no DMA transposes.
  - Qubit q <-> bit q of the state index, bit 0 = MSB. Shard over
    (q21,q22,q23) = idx % 8 on input; over (q0,q1,q2) on output.
  - F1: U1 (q0..6) applied with lhsT = state chunks (stationary) and
    rhs = packed U1^T -> output comes out transposed: partitions q7..13.
  - F2: U2 (q6..13, q6 2x2-blocked in free dim), same fused transpose ->
    partitions q14..20.
  - P3: U3 (q13..20, q13 blocked) as a plain matmul (partitions stay).
  - 4-slice AllToAll swaps (q0,q1,q2) <-> (q21,q22,q23) via DRAM bounce.
  - P4: U4 embedded on (q21,q22,q23,q17..20); bf16 output, host upcasts.
"""

import numpy as np
import ml_dtypes

import concourse.bass as bass
import concourse.bacc as bacc
import concourse.mybir as mybir
import concourse.tile as tile
from concourse.bass_utils import run_bass_kernel_spmd

F32 = mybir.dt.float32
BF16 = mybir.dt.bfloat16

N_CORES = 8
BF = ml_dtypes.bfloat16


# ---------------------------------------------------------------------------
# Host-side gate fusion (identical math to the reference, incl. the
# _apply_gate permutation quirk at q=1)
# ---------------------------------------------------------------------------

def _embed_gate(mat, qubits, group):
    g = len(group)
    k = len(qubits)
    pos = [group.index(q) for q in qubits]
    rest = [i for i in range(g) if i not in pos]
    U = np.zeros((1 << g, 1 << g), dtype=np.complex128)
    for r in range(1 << len(rest)):
        base = 0
        for bi, p in enumerate(rest):
            if (r >> (len(rest) - 1 - bi)) & 1:
                base |= 1 << (g - 1 - p)
        for a in range(1 << k):
            ia = base
            for bi, p in enumerate(pos):
                if (a >> (k - 1 - bi)) & 1:
                    ia |= 1 << (g - 1 - p)
            for b in range(1 << k):
                ib = base
                for bi, p in enumerate(pos):
                    if (b >> (k - 1 - bi)) & 1:
                        ib |= 1 << (g - 1 - p)
                U[ia, ib] = mat[a, b]
    return U


def _quirk_P():
    # reference._apply_gate on [1,2]: extra relabeling on qubits (0,1,2):
    # new (b0,b1,b2) = (old b2, old b0, old b1).
    P = np.zeros((8, 8), dtype=np.complex128)
    for b0 in range(2):
        for b1 in range(2):
            for b2 in range(2):
                P[(b2 << 2) | (b0 << 1) | b1, (b0 << 2) | (b1 << 1) | b2] = 1
    return P


def _fuse(ops, group):
    U = np.eye(1 << len(group), dtype=np.complex128)
    for mat, qb in ops:
        U = _embed_gate(mat, qb, group) @ U
    return U


def build_chunk_matrices(gates1, gates2):
    g1 = gates1[:, 0].astype(np.float64) + 1j * gates1[:, 1].astype(np.float64)
    g2 = gates2[:, 0].astype(np.float64) + 1j * gates2[:, 1].astype(np.float64)

    ops1 = [(g1[q], [q]) for q in range(0, 7)]
    ops1 += [(g2[0], [0, 1]), (g2[1], [1, 2]), (_quirk_P(), [0, 1, 2])]
    ops1 += [(g2[q], [q, q + 1]) for q in range(2, 6)]
    U1 = _fuse(ops1, list(range(0, 7)))

    ops2 = [(g1[q], [q]) for q in range(7, 14)]
    ops2 += [(g2[q], [q, q + 1]) for q in range(6, 13)]
    U2 = _fuse(ops2, list(range(6, 14)))  # q6 = MSB of the 256 index

    ops3 = [(g1[q], [q]) for q in range(14, 21)]
    ops3 += [(g2[q], [q, q + 1]) for q in range(13, 20)]
    U3 = _fuse(ops3, list(range(13, 21)))  # q13 = MSB

    ops4 = [(g1[q], [q]) for q in range(21, 24)]
    ops4 += [(g2[q], [q, q + 1]) for q in range(20, 23)]
    U4 = _fuse(ops4, [21, 22, 23, 17, 18, 19, 20])

    return U1, U2, U3, U4


def _pack_lhsT(U):
    """lhsT components for out = U @ x (complex): A = re(U)^T, B = im(U)^T,
    Bn = -im(U)^T, stacked (3, n, n) bf16."""
    return np.stack([U.real.T, U.imag.T, -U.imag.T]).astype(BF)


def build_weights(gates1, gates2):
    U1, U2, U3, U4 = build_chunk_matrices(gates1, gates2)

    # F1 rhs: psum[m, pl'*128+j] += sum_p x_pl[p,m] * w1[pl][p, pl'*128+j]
    w1 = np.zeros((2, 128, 256), dtype=np.float64)
    w1[0, :, :128] = U1.real.T
    w1[0, :, 128:] = U1.imag.T
    w1[1, :, :128] = -U1.imag.T
    w1[1, :, 128:] = U1.real.T

    # F2 rhs: col n = j'*256 + pl'*128 + t, t = (q13')(q7'..12')
    perm = np.array([((t & 63) << 1) | (t >> 6) for t in range(128)])
    U2v = U2.reshape(2, 128, 2, 128)[:, perm, :, :]  # [j', t, k, p]
    w2 = np.zeros((2, 2, 128, 512), dtype=np.float64)
    for k in range(2):
        M = U2v[:, :, k, :]  # [j', t, p]
        for jp in range(2):
            w2[0, k, :, jp * 256:jp * 256 + 128] = M[jp].real.T
            w2[0, k, :, jp * 256 + 128:jp * 256 + 256] = M[jp].imag.T
            w2[1, k, :, jp * 256:jp * 256 + 128] = -M[jp].imag.T
            w2[1, k, :, jp * 256 + 128:jp * 256 + 256] = M[jp].real.T

    # P3 lhsT blocks: U3 index (q13, q14..20); block (j=q13', k=q13)
    w3 = np.stack([
        np.stack([_pack_lhsT(U3[j * 128:(j + 1) * 128, k * 128:(k + 1) * 128])
                  for k in (0, 1)])
        for j in (0, 1)])

    return {
        "w1": w1.astype(BF),
        "w2": w2.astype(BF),
        "w3": w3,
        "w4": _pack_lhsT(U4),
    }


# ---------------------------------------------------------------------------
# Bass kernel builder
# ---------------------------------------------------------------------------

def build_nc():
    nc = bacc.Bacc()

    st = nc.declare_dram_parameter("state", [2, 1 << 21], BF16, isOutput=False)
    w1 = nc.declare_dram_parameter("w1", [2, 128, 256], BF16, isOutput=False)
    w2 = nc.declare_dram_parameter("w2", [2, 2, 128, 512], BF16, isOutput=False)
    w3 = nc.declare_dram_parameter("w3", [2, 2, 3, 128, 128], BF16, isOutput=False)
    w4 = nc.declare_dram_parameter("w4", [3, 128, 128], BF16, isOutput=False)
    out = nc.declare_dram_parameter("out", [2, 1 << 21], BF16, isOutput=True)

    # AllToAll bounce buffers, one per quarter v = (q3',q4').
    # Block for dest core b3: [128 rows = q14'..20',
    #                          1024 = (pl,j=q13',e=q5'6',r)]
    a2a_in = [nc.dram_tensor(f"a2a_in{v}", [8, 128, 1024], BF16)
              for v in range(4)]
    a2a_out = [nc.dram_tensor(f"a2a_out{v}", [8, 128, 1024], BF16)
               for v in range(4)]

    with tile.TileContext(nc, num_cores=N_CORES) as tc:
        with tc.tile_pool(name="state", bufs=1) as sp, \
             tc.tile_pool(name="wpool", bufs=1) as wp, \
             tc.tile_pool(name="mm", bufs=4, space="PSUM") as mmp, \
             tc.tile_pool(name="outp", bufs=4) as op:

            A = [sp.tile([128, 16384], BF16, tag=f"A{pl}", name=f"A{pl}")
                 for pl in (0, 1)]
            B = [sp.tile([128, 16384], BF16, tag=f"B{pl}", name=f"B{pl}")
                 for pl in (0, 1)]
            # stg has its own space (deferred F2 chunks still read B);
            # R aliases B[1] (readback is emitted after all F2 reads)
            stg = sp.tile([128, 16384], BF16, tag="stg", name="stg")
            Rt = B[1]

            # ---- weights (gpsimd queue so state load owns sync/scalar) ----
            w1t = []
            for pl in (0, 1):
                t = wp.tile([128, 256], BF16, tag=f"w1_{pl}")
                nc.gpsimd.dma_start(out=t[:], in_=w1[pl])
                w1t.append(t)
            w2t = [[None, None], [None, None]]
            for pl in (0, 1):
                for k in (0, 1):
                    t = wp.tile([128, 512], BF16, tag=f"w2_{pl}{k}")
                    nc.gpsimd.dma_start(out=t[:], in_=w2[pl, k])
                    w2t[pl][k] = t

            def load_w3(dram_ap3, name):
                ts = []
                for i in range(3):
                    t = wp.tile([128, 128], BF16, tag=f"{name}_{i}")
                    nc.gpsimd.dma_start(out=t[:], in_=dram_ap3[i])
                    ts.append(t)
                return ts

            w3t = [[load_w3(w3[j, k], f"w3_{j}{k}") for k in (0, 1)] for j in (0, 1)]
            w4t = load_w3(w4, "w4")

            # ---- state load: partitions (q0..q6), free (q7..q13)x(q14..q20)
            st_v = [st[pl].rearrange("(p f) -> p f", p=128) for pl in (0, 1)]
            for c in range(8):
                for pl in (0, 1):
                    eng = nc.sync if (c + pl) % 2 == 0 else nc.scalar
                    eng.dma_start(
                        out=A[pl][:, c * 2048:(c + 1) * 2048],
                        in_=st_v[pl][:, c * 2048:(c + 1) * 2048])

            # copy engines, rotated per call (only DVE/Act can read PSUM)
            cp_engines = [nc.vector.tensor_copy,
                          lambda out, in_: nc.scalar.copy(out=out, in_=in_)]
            cp_i = [0]

            def copy(out_ap, in_ap):
                cp_engines[cp_i[0] % 2](out_ap, in_ap)
                cp_i[0] += 1

            # ---- F1: U1 fused with transpose (q0..6) -> (q7..13) ----
            # Host pre-permutes shard to free = (b=q14..20)(a=q7..13), so
            # lhsT = A[pl][:, b*128:+128] is contiguous (FWL) and chunk b
            # only needs load-block b//16. psum quad [128,1024] = 4 chunks.
            def f1_quad(b0):
                ps = mmp.tile([128, 1024], F32, tag="mm", name=f"f1_{b0}")
                for i in range(4):
                    b = b0 + i
                    nc.tensor.matmul(ps[:, i * 256:i * 256 + 256],
                                     A[0][:, b * 128:(b + 1) * 128],
                                     w1t[0][:], start=True, stop=False)
                    nc.tensor.matmul(ps[:, i * 256:i * 256 + 256],
                                     A[1][:, b * 128:(b + 1) * 128],
                                     w1t[1][:], start=False, stop=True)
                ps_v = ps[:].rearrange("p (i pl j) -> p i pl j", i=4, pl=2)
                for pl in (0, 1):
                    copy(B[pl][:, b0 * 128:b0 * 128 + 512]
                         .rearrange("p (i j) -> p i j", i=4),
                         ps_v[:, :, pl, :])

            for b0 in range(0, 128, 4):
                f1_quad(b0)

            # ---- F2: U2 fused with transpose (q7..13) -> (q14..20) ----
            # lhsT = B chunk [p=q7..13, m=b=q14..20] for fixed j=(c,k);
            # psum_c[b, (j' 2)(pl' 2)(t 128)] -> A[pl'][b, (c*2+j')*128 + t]
            B_v = [B[pl][:].rearrange("p (b j) -> p j b", j=128) for pl in (0, 1)]

            def f2_pair(c0):
                # two adjacent c-chunks share one [128,1024] psum (2 banks)
                ps = mmp.tile([128, 1024], F32, tag="mm", name=f"f2_{c0}")
                for i in (0, 1):
                    c = c0 + i
                    first = True
                    for k in (0, 1):
                        for pl in (0, 1):
                            nc.tensor.matmul(ps[:, i * 512:i * 512 + 512],
                                             B_v[pl][:, c * 2 + k, :],
                                             w2t[pl][k][:], start=first,
                                             stop=(k == 1 and pl == 1))
                            first = False
                ps_v = ps[:].rearrange("p (i j pl t) -> p i j pl t",
                                       i=2, j=2, pl=2)
                for pl in (0, 1):
                    copy(A[pl][:, c0 * 256:c0 * 256 + 512]
                         .rearrange("p (i j t) -> p i j t", i=2, j=2),
                         ps_v[:, :, :, pl, :])

            # F2 pairs are emitted interleaved with P3 quarters below:
            # quarter (q3,q4) needs only pairs c0 = b3*8 + q3*4 + q4*2.

            # ---- P3: U3 (2x2 blocks over k=q13'); partitions (q14..q20) ----
            # C = A: free = (g=(c,j') 128)(k=q13' 2)(r=q7'..12' 64)
            C_v = [A[pl][:].rearrange("p (g k r) -> p g k r", g=128, k=2)
                   for pl in (0, 1)]
            # stg (=B[0]): free = (b3 8)(q4 2)(pl 2)(j 2)(e 4)(r 64)
            stg_v = stg[:].rearrange("p (b3 q4 pl j e r) -> p b3 q4 pl j e r",
                                     b3=8, q4=2, pl=2, j=2, e=4)

            def p3_piece(b3, q3, q4):
                g0 = b3 * 16 + q3 * 8 + q4 * 4
                xs = {(pl, k): C_v[pl][:, g0:g0 + 4, k, :] for pl in (0, 1)
                      for k in (0, 1)}
                # one [128,1024] psum: (pl 2)(j 2)(e 4)(r 64)
                ps = mmp.tile([128, 1024], F32, tag="mm",
                              name=f"p3_{b3}_{q3}_{q4}")
                pj = [[ps[:, pl * 512 + j * 256:pl * 512 + (j + 1) * 256]
                       for j in (0, 1)] for pl in (0, 1)]
                # start=True clears has_written for the WHOLE bank, so each
                # psum slice's accumulation group must run uninterleaved
                # w.r.t. the other slice sharing its bank.
                for pl in (0, 1):
                    for j in (0, 1):
                        for k in (0, 1):
                            A3, B3, Bn3 = w3t[j][k]
                            if pl == 0:
                                nc.tensor.matmul(pj[0][j], A3[:], xs[(0, k)],
                                                 start=(k == 0), stop=False)
                                nc.tensor.matmul(pj[0][j], Bn3[:], xs[(1, k)],
                                                 start=False, stop=(k == 1))
                            else:
                                nc.tensor.matmul(pj[1][j], A3[:], xs[(1, k)],
                                                 start=(k == 0), stop=False)
                                nc.tensor.matmul(pj[1][j], B3[:], xs[(0, k)],
                                                 start=False, stop=(k == 1))
                for pl in (0, 1):
                    copy(stg_v[:, b3, q4, pl],
                         ps[:, pl * 512:(pl + 1) * 512]
                         .rearrange("p (j e r) -> p j e r", j=2, e=4))

            def stage_dma(b3, q3, q4):
                eng = nc.sync if b3 % 2 == 0 else nc.scalar
                eng.dma_start(out=a2a_in[q3 * 2 + q4][b3],
                              in_=stg_v[:, b3, q4].rearrange(
                                  "p pl j e r -> p (pl j e r)"))

            def collective(v):
                nc.gpsimd.collective_compute(
                    "AllToAll",
                    mybir.AluOpType.bypass,
                    replica_groups=[list(range(N_CORES))],
                    ins=[a2a_in[v].ap().opt()],
                    outs=[a2a_out[v].ap().opt()],
                )

            # ---- tail: readback + P4 + store, per quarter v ----
            # R (=B[1] halves): free = (pl 2)(w3 8)(c 512=(j,e,r))
            ov = [out[pl].rearrange("(p f) -> p f", p=128) for pl in (0, 1)]

            def readback(v):
                R = Rt[:, (v % 2) * 8192:(v % 2) * 8192 + 8192]
                for h3 in range(8):
                    eng = nc.sync if h3 % 2 == 0 else nc.scalar
                    eng.dma_start(
                        out=R[h3 * 16:(h3 + 1) * 16, :]
                            .rearrange("m (pl w3 c) -> m pl w3 c", pl=2, w3=8),
                        in_=a2a_out[v][h3].rearrange(
                            "(w3 m) (pl c) -> m pl w3 c", m=16, pl=2))
                return R

            def p4_chunk(v, R, w3c):
                A4, B4, Bn4 = w4t
                rre = R[:, w3c * 512:(w3c + 1) * 512]
                rim = R[:, 4096 + w3c * 512:4096 + (w3c + 1) * 512]
                ps = mmp.tile([128, 1024], F32, tag="mm", name=f"p4_{v}_{w3c}")
                pre = ps[:, 0:512]
                pim = ps[:, 512:1024]
                nc.tensor.matmul(pre, A4[:], rre, start=True, stop=False)
                nc.tensor.matmul(pim, A4[:], rim, start=True, stop=False)
                nc.tensor.matmul(pim, B4[:], rre, start=False, stop=True)
                nc.tensor.matmul(pre, Bn4[:], rim, start=False, stop=True)
                c0 = v * 4096 + w3c * 512
                ot = op.tile([128, 1024], BF16, tag="p4out")
                copy(ot[:], ps[:])
                for pl in (0, 1):
                    eng = nc.sync if (w3c + pl) % 2 == 0 else nc.scalar
                    eng.dma_start(out=ov[pl][:, c0:c0 + 512],
                                  in_=ot[:, pl * 512:(pl + 1) * 512])

            # ---- emission ----
            for q3 in (0, 1):
                for q4 in (0, 1):
                    for b3 in range(8):
                        f2_pair(b3 * 8 + q3 * 4 + q4 * 2)
                    for b3 in range(8):
                        p3_piece(b3, q3, q4)
                        stage_dma(b3, q3, q4)
                    collective(q3 * 2 + q4)
            for v in range(4):
                R = readback(v)
                for w3c in range(8):
                    p4_chunk(v, R, w3c)

    return nc


# ---------------------------------------------------------------------------
# Host wrapper
# ---------------------------------------------------------------------------

TRACE = False
LAST_EXEC_NS = None
LAST_RESULTS = None


def kernel(state, gates1, gates2):
    global LAST_EXEC_NS, LAST_RESULTS
    state = np.asarray(state, dtype=np.float32)
    weights = build_weights(np.asarray(gates1, dtype=np.float32),
                            np.asarray(gates2, dtype=np.float32))

    # shard over (q21,q22,q23) = index mod 8, cast to bf16.
    # Local free layout is (q14..20)(q7..13) — a/b swapped so F1's lhsT
    # chunks are contiguous in SBUF.
    shards = state.reshape(2, 1 << 21, 8).transpose(2, 0, 1)
    shards = np.ascontiguousarray(
        shards.reshape(8, 2, 128, 128, 128).swapaxes(3, 4)
    ).reshape(8, 2, 1 << 21).astype(BF)

    nc = build_nc()
    if not nc.is_finalized():
        nc.finalize()
    in_maps = [dict(weights, state=shards[d]) for d in range(N_CORES)]
    res = run_bass_kernel_spmd(nc, in_maps, core_ids=list(range(N_CORES)),
                               trace=TRACE)
    LAST_EXEC_NS = res.exec_time_ns
    LAST_RESULTS = res

    return unshard([res.results[d]["out"] for d in range(N_CORES)])


def unshard(outs):
    # core d holds (q0,q1,q2) = d;
    # local layout: [pl][p = (q21,q22,q23,q17,q18,q19,q20)]
    #               [f = (q3,q4)(q14,q15,q16)(q13)(q5,q6)(q7..q12)]
    full = np.empty((2, 8) + (2,) * 21, dtype=np.float32)
    bits = [21, 22, 23, 17, 18, 19, 20, 3, 4, 14, 15, 16, 13, 5, 6,
            7, 8, 9, 10, 11, 12]
    perm = [bits.index(3 + i) for i in range(21)]
    for d in range(N_CORES):
        od = np.asarray(outs[d]).astype(np.float32).reshape((2,) + (2,) * 21)
        full[:, d] = np.transpose(od, [0] + [1 + p for p in perm])
    return full.reshape(2, 1 << 24)


if __name__ == "__main__":
    rng = np.random.default_rng(0)
    state = rng.standard_normal((2, 1 << 24)).astype(np.float32)
    g1 = rng.standard_normal((24, 2, 2, 2)).astype(np.float32)
    g2 = rng.standard_normal((23, 2, 4, 4)).astype(np.float32)
    out = kernel(state, g1, g2)
    print(out.shape, out.dtype)


# revision 25
# speedup vs baseline: 1.0376x; 1.0376x over previous
"""Trainium2 Bass kernel for nn_Circuit_19275813225041 (v3).

24-qubit state-vector simulation: one layer of single-qubit gates on every
qubit, then a ladder of two-qubit gates on neighboring pairs (q, q+1),
q = 0..22, on a 2^24 complex state stored as (2, 2^24) float32 (re, im).

v3 strategy (8 NeuronCores): fused matmul-transposes, no DMA transposes.
  - Qubit q <-> bit q of the state index, bit 0 = MSB. Shard over
    (q21,q22,q23) = idx % 8 on input; over (q0,q1,q2) on output.
  - F1: U1 (q0..6) applied with lhsT = state chunks (stationary) and
    rhs = packed U1^T -> output comes out transposed: partitions q7..13.
  - F2: U2 (q6..13, q6 2x2-blocked in free dim), same fused transpose ->
    partitions q14..20.
  - P3: U3 (q13..20, q13 blocked) as a plain matmul (partitions stay).
  - 4-slice AllToAll swaps (q0,q1,q2) <-> (q21,q22,q23) via DRAM bounce.
  - P4: U4 embedded on (q21,q22,q23,q17..20); bf16 output, host upcasts.
"""

import numpy as np
import ml_dtypes

import concourse.bass as bass
import concourse.bacc as bacc
import concourse.mybir as mybir
import concourse.tile as tile
from concourse.bass_utils import run_bass_kernel_spmd

F32 = mybir.dt.float32
BF16 = mybir.dt.bfloat16

N_CORES = 8
BF = ml_dtypes.bfloat16


# ---------------------------------------------------------------------------
# Host-side gate fusion (identical math to the reference, incl. the
# _apply_gate permutation quirk at q=1)
# ---------------------------------------------------------------------------

def _embed_gate(mat, qubits, group):
    g = len(group)
    k = len(qubits)
    pos = [group.index(q) for q in qubits]
    rest = [i for i in range(g) if i not in pos]
    U = np.zeros((1 << g, 1 << g), dtype=np.complex128)
    for r in range(1 << len(rest)):
        base = 0
        for bi, p in enumerate(rest):
            if (r >> (len(rest) - 1 - bi)) & 1:
                base |= 1 << (g - 1 - p)
        for a in range(1 << k):
            ia = base
            for bi, p in enumerate(pos):
                if (a >> (k - 1 - bi)) & 1:
                    ia |= 1 << (g - 1 - p)
            for b in range(1 << k):
                ib = base
                for bi, p in enumerate(pos):
                    if (b >> (k - 1 - bi)) & 1:
                        ib |= 1 << (g - 1 - p)
                U[ia, ib] = mat[a, b]
    return U


def _quirk_P():
    # reference._apply_gate on [1,2]: extra relabeling on qubits (0,1,2):
    # new (b0,b1,b2) = (old b2, old b0, old b1).
    P = np.zeros((8, 8), dtype=np.complex128)
    for b0 in range(2):
        for b1 in range(2):
            for b2 in range(2):
                P[(b2 << 2) | (b0 << 1) | b1, (b0 << 2) | (b1 << 1) | b2] = 1
    return P


def _fuse(ops, group):
    U = np.eye(1 << len(group), dtype=np.complex128)
    for mat, qb in ops:
        U = _embed_gate(mat, qb, group) @ U
    return U


def build_chunk_matrices(gates1, gates2):
    g1 = gates1[:, 0].astype(np.float64) + 1j * gates1[:, 1].astype(np.float64)
    g2 = gates2[:, 0].astype(np.float64) + 1j * gates2[:, 1].astype(np.float64)

    ops1 = [(g1[q], [q]) for q in range(0, 7)]
    ops1 += [(g2[0], [0, 1]), (g2[1], [1, 2]), (_quirk_P(), [0, 1, 2])]
    ops1 += [(g2[q], [q, q + 1]) for q in range(2, 6)]
    U1 = _fuse(ops1, list(range(0, 7)))

    ops2 = [(g1[q], [q]) for q in range(7, 14)]
    ops2 += [(g2[q], [q, q + 1]) for q in range(6, 13)]
    U2 = _fuse(ops2, list(range(6, 14)))  # q6 = MSB of the 256 index

    ops3 = [(g1[q], [q]) for q in range(14, 21)]
    ops3 += [(g2[q], [q, q + 1]) for q in range(13, 20)]
    U3 = _fuse(ops3, list(range(13, 21)))  # q13 = MSB

    ops4 = [(g1[q], [q]) for q in range(21, 24)]
    ops4 += [(g2[q], [q, q + 1]) for q in range(20, 23)]
    U4 = _fuse(ops4, [21, 22, 23, 17, 18, 19, 20])

    return U1, U2, U3, U4


def _pack_lhsT(U):
    """lhsT components for out = U @ x (complex): A = re(U)^T, B = im(U)^T,
    Bn = -im(U)^T, stacked (3, n, n) bf16."""
    return np.stack([U.real.T, U.imag.T, -U.imag.T]).astype(BF)


def build_weights(gates1, gates2):
    U1, U2, U3, U4 = build_chunk_matrices(gates1, gates2)

    # F1 rhs: psum[m, pl'*128+j] += sum_p x_pl[p,m] * w1[pl][p, pl'*128+j]
    w1 = np.zeros((2, 128, 256), dtype=np.float64)
    w1[0, :, :128] = U1.real.T
    w1[0, :, 128:] = U1.imag.T
    w1[1, :, :128] = -U1.imag.T
    w1[1, :, 128:] = U1.real.T

    # F2 rhs: col n = j'*256 + pl'*128 + t, t = (q13')(q7'..12')
    perm = np.array([((t & 63) << 1) | (t >> 6) for t in range(128)])
    U2v = U2.reshape(2, 128, 2, 128)[:, perm, :, :]  # [j', t, k, p]
    w2 = np.zeros((2, 2, 128, 512), dtype=np.float64)
    for k in range(2):
        M = U2v[:, :, k, :]  # [j', t, p]
        for jp in range(2):
            w2[0, k, :, jp * 256:jp * 256 + 128] = M[jp].real.T
            w2[0, k, :, jp * 256 + 128:jp * 256 + 256] = M[jp].imag.T
            w2[1, k, :, jp * 256:jp * 256 + 128] = -M[jp].imag.T
            w2[1, k, :, jp * 256 + 128:jp * 256 + 256] = M[jp].real.T

    # P3 lhsT blocks: U3 index (q13, q14..20); block (j=q13', k=q13)
    w3 = np.stack([
        np.stack([_pack_lhsT(U3[j * 128:(j + 1) * 128, k * 128:(k + 1) * 128])
                  for k in (0, 1)])
        for j in (0, 1)])

    return {
        "w1": w1.astype(BF),
        "w2": w2.astype(BF),
        "w3": w3,
        "w4": _pack_lhsT(U4),
    }


# ---------------------------------------------------------------------------
# Bass kernel builder
# ---------------------------------------------------------------------------

def build_nc():
    nc = bacc.Bacc()

    st = nc.declare_dram_parameter("state", [2, 1 << 21], BF16, isOutput=False)
    w1 = nc.declare_dram_parameter("w1", [2, 128, 256], BF16, isOutput=False)
    w2 = nc.declare_dram_parameter("w2", [2, 2, 128, 512], BF16, isOutput=False)
    w3 = nc.declare_dram_parameter("w3", [2, 2, 3, 128, 128], BF16, isOutput=False)
    w4 = nc.declare_dram_parameter("w4", [3, 128, 128], BF16, isOutput=False)
    out = nc.declare_dram_parameter("out", [2, 1 << 21], BF16, isOutput=True)

    # AllToAll bounce buffers, one per quarter v = (q3',q4').
    # Block for dest core b3: [128 rows = q14'..20',
    #                          1024 = (pl,j=q13',e=q5'6',r)]
    a2a_in = [nc.dram_tensor(f"a2a_in{v}", [8, 128, 1024], BF16)
              for v in range(4)]
    a2a_out = [nc.dram_tensor(f"a2a_out{v}", [8, 128, 1024], BF16)
               for v in range(4)]

    with tile.TileContext(nc, num_cores=N_CORES) as tc:
        with tc.tile_pool(name="state", bufs=1) as sp, \
             tc.tile_pool(name="wpool", bufs=1) as wp, \
             tc.tile_pool(name="mm", bufs=4, space="PSUM") as mmp, \
             tc.tile_pool(name="outp", bufs=4) as op:

            A = [sp.tile([128, 16384], BF16, tag=f"A{pl}", name=f"A{pl}")
                 for pl in (0, 1)]
            B = [sp.tile([128, 16384], BF16, tag=f"B{pl}", name=f"B{pl}")
                 for pl in (0, 1)]
            # stg has its own space (deferred F2 chunks still read B);
            # R aliases B[1] (readback is emitted after all F2 reads)
            stg = sp.tile([128, 16384], BF16, tag="stg", name="stg")
            Rt = B[1]

            # ---- weights (gpsimd queue so state load owns sync/scalar) ----
            w1t = []
            for pl in (0, 1):
                t = wp.tile([128, 256], BF16, tag=f"w1_{pl}")
                nc.gpsimd.dma_start(out=t[:], in_=w1[pl])
                w1t.append(t)
            w2t = [[None, None], [None, None]]
            for pl in (0, 1):
                for k in (0, 1):
                    t = wp.tile([128, 512], BF16, tag=f"w2_{pl}{k}")
                    nc.gpsimd.dma_start(out=t[:], in_=w2[pl, k])
                    w2t[pl][k] = t

            def load_w3(dram_ap3, name):
                ts = []
                for i in range(3):
                    t = wp.tile([128, 128], BF16, tag=f"{name}_{i}")
                    nc.gpsimd.dma_start(out=t[:], in_=dram_ap3[i])
                    ts.append(t)
                return ts

            w3t = [[load_w3(w3[j, k], f"w3_{j}{k}") for k in (0, 1)] for j in (0, 1)]
            w4t = load_w3(w4, "w4")

            # ---- state load: partitions (q0..q6), free (q7..q13)x(q14..q20)
            st_v = [st[pl].rearrange("(p f) -> p f", p=128) for pl in (0, 1)]
            for c in range(8):
                for pl in (0, 1):
                    eng = nc.sync if (c + pl) % 2 == 0 else nc.scalar
                    eng.dma_start(
                        out=A[pl][:, c * 2048:(c + 1) * 2048],
                        in_=st_v[pl][:, c * 2048:(c + 1) * 2048])

            # copy engines, rotated per call (only DVE/Act can read PSUM)
            cp_engines = [nc.vector.tensor_copy,
                          lambda out, in_: nc.scalar.copy(out=out, in_=in_)]
            cp_i = [0]

            def copy(out_ap, in_ap):
                cp_engines[cp_i[0] % 2](out_ap, in_ap)
                cp_i[0] += 1

            # ---- F1: U1 fused with transpose (q0..6) -> (q7..13) ----
            # Host pre-permutes shard to free = (b=q14..20)(a=q7..13), so
            # lhsT = A[pl][:, b*128:+128] is contiguous (FWL) and chunk b
            # only needs load-block b//16. psum quad [128,1024] = 4 chunks.
            def f1_quad(b0):
                ps = mmp.tile([128, 1024], F32, tag="mm", name=f"f1_{b0}")
                for i in range(4):
                    b = b0 + i
                    nc.tensor.matmul(ps[:, i * 256:i * 256 + 256],
                                     A[0][:, b * 128:(b + 1) * 128],
                                     w1t[0][:], start=True, stop=False)
                    nc.tensor.matmul(ps[:, i * 256:i * 256 + 256],
                                     A[1][:, b * 128:(b + 1) * 128],
                                     w1t[1][:], start=False, stop=True)
                ps_v = ps[:].rearrange("p (i pl j) -> p i pl j", i=4, pl=2)
                for pl in (0, 1):
                    copy(B[pl][:, b0 * 128:b0 * 128 + 512]
                         .rearrange("p (i j) -> p i j", i=4),
                         ps_v[:, :, pl, :])

            for b0 in range(0, 128, 4):
                f1_quad(b0)

            # ---- F2: U2 fused with transpose (q7..13) -> (q14..20) ----
            # lhsT = B chunk [p=q7..13, m=b=q14..20] for fixed j=(c,k);
            # psum_c[b, (j' 2)(pl' 2)(t 128)] -> A[pl'][b, (c*2+j')*128 + t]
            B_v = [B[pl][:].rearrange("p (b j) -> p j b", j=128) for pl in (0, 1)]

            def f2_pair(c0):
                # two adjacent c-chunks share one [128,1024] psum (2 banks)
                ps = mmp.tile([128, 1024], F32, tag="mm", name=f"f2_{c0}")
                for i in (0, 1):
                    c = c0 + i
                    first = True
                    for k in (0, 1):
                        for pl in (0, 1):
                            nc.tensor.matmul(ps[:, i * 512:i * 512 + 512],
                                             B_v[pl][:, c * 2 + k, :],
                                             w2t[pl][k][:], start=first,
                                             stop=(k == 1 and pl == 1))
                            first = False
                ps_v = ps[:].rearrange("p (i j pl t) -> p i j pl t",
                                       i=2, j=2, pl=2)
                for pl in (0, 1):
                    copy(A[pl][:, c0 * 256:c0 * 256 + 512]
                         .rearrange("p (i j t) -> p i j t", i=2, j=2),
                         ps_v[:, :, :, pl, :])

            # F2 pairs are emitted interleaved with P3 quarters below:
            # quarter (q3,q4) needs only pairs c0 = b3*8 + q3*4 + q4*2.

            # ---- P3: U3 (2x2 blocks over k=q13'); partitions (q14..q20) ----
            # C = A: free = (g=(c,j') 128)(k=q13' 2)(r=q7'..12' 64)
            C_v = [A[pl][:].rearrange("p (g k r) -> p g k r", g=128, k=2)
                   for pl in (0, 1)]
            # stg (=B[0]): free = (b3 8)(q4 2)(pl 2)(j 2)(e 4)(r 64)
            stg_v = stg[:].rearrange("p (b3 q4 pl j e r) -> p b3 q4 pl j e r",
                                     b3=8, q4=2, pl=2, j=2, e=4)

            def p3_piece(b3, q3, q4):
                g0 = b3 * 16 + q3 * 8 + q4 * 4
                xs = {(pl, k): C_v[pl][:, g0:g0 + 4, k, :] for pl in (0, 1)
                      for k in (0, 1)}
                # one [128,1024] psum: (pl 2)(j 2)(e 4)(r 64)
                ps = mmp.tile([128, 1024], F32, tag="mm",
                              name=f"p3_{b3}_{q3}_{q4}")
                pj = [[ps[:, pl * 512 + j * 256:pl * 512 + (j + 1) * 256]
                       for j in (0, 1)] for pl in (0, 1)]
                # start=True clears has_written for the WHOLE bank, so each
                # psum slice's accumulation group must run uninterleaved
                # w.r.t. the other slice sharing its bank.
                for pl in (0, 1):
                    for j in (0, 1):
                        for k in (0, 1):
                            A3, B3, Bn3 = w3t[j][k]
                            if pl == 0:
                                nc.tensor.matmul(pj[0][j], A3[:], xs[(0, k)],
                                                 start=(k == 0), stop=False)
                                nc.tensor.matmul(pj[0][j], Bn3[:], xs[(1, k)],
                                                 start=False, stop=(k == 1))
                            else:
                                nc.tensor.matmul(pj[1][j], A3[:], xs[(1, k)],
                                                 start=(k == 0), stop=False)
                                nc.tensor.matmul(pj[1][j], B3[:], xs[(0, k)],
                                                 start=False, stop=(k == 1))
                copy(stg_v[:, b3, q4], ps[:].rearrange(
                    "p (pl j e r) -> p pl j e r", pl=2, j=2, e=4))

            def stage_dma(b3, q3, q4):
                eng = nc.sync if b3 % 2 == 0 else nc.scalar
                eng.dma_start(out=a2a_in[q3 * 2 + q4][b3],
                              in_=stg_v[:, b3, q4].rearrange(
                                  "p pl j e r -> p (pl j e r)"))

            def collective(v):
                nc.gpsimd.collective_compute(
                    "AllToAll",
                    mybir.AluOpType.bypass,
                    replica_groups=[list(range(N_CORES))],
                    ins=[a2a_in[v].ap().opt()],
                    outs=[a2a_out[v].ap().opt()],
                )

            # ---- tail: readback + P4 + store, per quarter v ----
            # R (=B[1] halves): free = (pl 2)(w3 8)(c 512=(j,e,r))
            ov = [out[pl].rearrange("(p f) -> p f", p=128) for pl in (0, 1)]

            def readback(v):
                # sync queue only — P4 out-DMAs own the scalar queue, so a
                # blocked readback never stalls them (and vice versa)
                R = Rt[:, (v % 2) * 8192:(v % 2) * 8192 + 8192]
                for h3 in range(8):
                    eng = nc.sync
                    eng.dma_start(
                        out=R[h3 * 16:(h3 + 1) * 16, :]
                            .rearrange("m (pl w3 c) -> m pl w3 c", pl=2, w3=8),
                        in_=a2a_out[v][h3].rearrange(
                            "(w3 m) (pl c) -> m pl w3 c", m=16, pl=2))
                return R

            def p4_chunk(v, R, w3c):
                A4, B4, Bn4 = w4t
                rre = R[:, w3c * 512:(w3c + 1) * 512]
                rim = R[:, 4096 + w3c * 512:4096 + (w3c + 1) * 512]
                ps = mmp.tile([128, 1024], F32, tag="mm", name=f"p4_{v}_{w3c}")
                pre = ps[:, 0:512]
                pim = ps[:, 512:1024]
                nc.tensor.matmul(pre, A4[:], rre, start=True, stop=False)
                nc.tensor.matmul(pim, A4[:], rim, start=True, stop=False)
                nc.tensor.matmul(pim, B4[:], rre, start=False, stop=True)
                nc.tensor.matmul(pre, Bn4[:], rim, start=False, stop=True)
                c0 = v * 4096 + w3c * 512
                ot = op.tile([128, 1024], BF16, tag="p4out")
                copy(ot[:], ps[:])
                for pl in (0, 1):
                    nc.scalar.dma_start(out=ov[pl][:, c0:c0 + 512],
                                        in_=ot[:, pl * 512:(pl + 1) * 512])

            # ---- emission ----
            for q3 in (0, 1):
                for q4 in (0, 1):
                    for b3 in range(8):
                        f2_pair(b3 * 8 + q3 * 4 + q4 * 2)
                    for b3 in range(8):
                        p3_piece(b3, q3, q4)
                        stage_dma(b3, q3, q4)
                    collective(q3 * 2 + q4)
            for v in range(4):
                R = readback(v)
                for w3c in range(8):
                    p4_chunk(v, R, w3c)

    return nc


# ---------------------------------------------------------------------------
# Host wrapper
# ---------------------------------------------------------------------------

TRACE = False
LAST_EXEC_NS = None
LAST_RESULTS = None


def kernel(state, gates1, gates2):
    global LAST_EXEC_NS, LAST_RESULTS
    state = np.asarray(state, dtype=np.float32)
    weights = build_weights(np.asarray(gates1, dtype=np.float32),
                            np.asarray(gates2, dtype=np.float32))

    # shard over (q21,q22,q23) = index mod 8, cast to bf16.
    # Local free layout is (q14..20)(q7..13) — a/b swapped so F1's lhsT
    # chunks are contiguous in SBUF.
    shards = state.reshape(2, 1 << 21, 8).transpose(2, 0, 1)
    shards = np.ascontiguousarray(
        shards.reshape(8, 2, 128, 128, 128).swapaxes(3, 4)
    ).reshape(8, 2, 1 << 21).astype(BF)

    nc = build_nc()
    if not nc.is_finalized():
        nc.finalize()
    in_maps = [dict(weights, state=shards[d]) for d in range(N_CORES)]
    res = run_bass_kernel_spmd(nc, in_maps, core_ids=list(range(N_CORES)),
                               trace=TRACE)
    LAST_EXEC_NS = res.exec_time_ns
    LAST_RESULTS = res

    return unshard([res.results[d]["out"] for d in range(N_CORES)])


def unshard(outs):
    # core d holds (q0,q1,q2) = d;
    # local layout: [pl][p = (q21,q22,q23,q17,q18,q19,q20)]
    #               [f = (q3,q4)(q14,q15,q16)(q13)(q5,q6)(q7..q12)]
    full = np.empty((2, 8) + (2,) * 21, dtype=np.float32)
    bits = [21, 22, 23, 17, 18, 19, 20, 3, 4, 14, 15, 16, 13, 5, 6,
            7, 8, 9, 10, 11, 12]
    perm = [bits.index(3 + i) for i in range(21)]
    for d in range(N_CORES):
        od = np.asarray(outs[d]).astype(np.float32).reshape((2,) + (2,) * 21)
        full[:, d] = np.transpose(od, [0] + [1 + p for p in perm])
    return full.reshape(2, 1 << 24)


if __name__ == "__main__":
    rng = np.random.default_rng(0)
    state = rng.standard_normal((2, 1 << 24)).astype(np.float32)
    g1 = rng.standard_normal((24, 2, 2, 2)).astype(np.float32)
    g2 = rng.standard_normal((23, 2, 4, 4)).astype(np.float32)
    out = kernel(state, g1, g2)
    print(out.shape, out.dtype)


# revision 30
# speedup vs baseline: 1.0758x; 1.0368x over previous
"""Trainium2 Bass kernel for nn_Circuit_19275813225041 (v3).

24-qubit state-vector simulation: one layer of single-qubit gates on every
qubit, then a ladder of two-qubit gates on neighboring pairs (q, q+1),
q = 0..22, on a 2^24 complex state stored as (2, 2^24) float32 (re, im).

v3 strategy (8 NeuronCores): fused matmul-transposes, no DMA transposes.
  - Qubit q <-> bit q of the state index, bit 0 = MSB. Shard over
    (q21,q22,q23) = idx % 8 on input; over (q0,q1,q2) on output.
  - F1: U1 (q0..6) applied with lhsT = state chunks (stationary) and
    rhs = packed U1^T -> output comes out transposed: partitions q7..13.
  - F2: U2 (q6..13, q6 2x2-blocked in free dim), same fused transpose ->
    partitions q14..20.
  - P3: U3 (q13..20, q13 blocked) as a plain matmul (partitions stay).
  - 4-slice AllToAll swaps (q0,q1,q2) <-> (q21,q22,q23) via DRAM bounce.
  - P4: U4 embedded on (q21,q22,q23,q17..20); bf16 output, host upcasts.
"""

import numpy as np
import ml_dtypes

import concourse.bass as bass
import concourse.bacc as bacc
import concourse.mybir as mybir
import concourse.tile as tile
from concourse.bass_utils import run_bass_kernel_spmd

F32 = mybir.dt.float32
BF16 = mybir.dt.bfloat16

N_CORES = 8
BF = ml_dtypes.bfloat16


# ---------------------------------------------------------------------------
# Host-side gate fusion (identical math to the reference, incl. the
# _apply_gate permutation quirk at q=1)
# ---------------------------------------------------------------------------

def _embed_gate(mat, qubits, group):
    g = len(group)
    k = len(qubits)
    pos = [group.index(q) for q in qubits]
    rest = [i for i in range(g) if i not in pos]
    U = np.zeros((1 << g, 1 << g), dtype=np.complex128)
    for r in range(1 << len(rest)):
        base = 0
        for bi, p in enumerate(rest):
            if (r >> (len(rest) - 1 - bi)) & 1:
                base |= 1 << (g - 1 - p)
        for a in range(1 << k):
            ia = base
            for bi, p in enumerate(pos):
                if (a >> (k - 1 - bi)) & 1:
                    ia |= 1 << (g - 1 - p)
            for b in range(1 << k):
                ib = base
                for bi, p in enumerate(pos):
                    if (b >> (k - 1 - bi)) & 1:
                        ib |= 1 << (g - 1 - p)
                U[ia, ib] = mat[a, b]
    return U


def _quirk_P():
    # reference._apply_gate on [1,2]: extra relabeling on qubits (0,1,2):
    # new (b0,b1,b2) = (old b2, old b0, old b1).
    P = np.zeros((8, 8), dtype=np.complex128)
    for b0 in range(2):
        for b1 in range(2):
            for b2 in range(2):
                P[(b2 << 2) | (b0 << 1) | b1, (b0 << 2) | (b1 << 1) | b2] = 1
    return P


def _fuse(ops, group):
    U = np.eye(1 << len(group), dtype=np.complex128)
    for mat, qb in ops:
        U = _embed_gate(mat, qb, group) @ U
    return U


def build_chunk_matrices(gates1, gates2):
    g1 = gates1[:, 0].astype(np.float64) + 1j * gates1[:, 1].astype(np.float64)
    g2 = gates2[:, 0].astype(np.float64) + 1j * gates2[:, 1].astype(np.float64)

    ops1 = [(g1[q], [q]) for q in range(0, 7)]
    ops1 += [(g2[0], [0, 1]), (g2[1], [1, 2]), (_quirk_P(), [0, 1, 2])]
    ops1 += [(g2[q], [q, q + 1]) for q in range(2, 6)]
    U1 = _fuse(ops1, list(range(0, 7)))

    ops2 = [(g1[q], [q]) for q in range(7, 14)]
    ops2 += [(g2[q], [q, q + 1]) for q in range(6, 13)]
    U2 = _fuse(ops2, list(range(6, 14)))  # q6 = MSB of the 256 index

    ops3 = [(g1[q], [q]) for q in range(14, 21)]
    ops3 += [(g2[q], [q, q + 1]) for q in range(13, 20)]
    U3 = _fuse(ops3, list(range(13, 21)))  # q13 = MSB

    ops4 = [(g1[q], [q]) for q in range(21, 24)]
    ops4 += [(g2[q], [q, q + 1]) for q in range(20, 23)]
    U4 = _fuse(ops4, [21, 22, 23, 17, 18, 19, 20])

    return U1, U2, U3, U4


def _pack_lhsT(U):
    """lhsT components for out = U @ x (complex): A = re(U)^T, B = im(U)^T,
    Bn = -im(U)^T, stacked (3, n, n) bf16."""
    return np.stack([U.real.T, U.imag.T, -U.imag.T]).astype(BF)


def build_weights(gates1, gates2):
    U1, U2, U3, U4 = build_chunk_matrices(gates1, gates2)

    # F1 rhs: psum[m, pl'*128+j] += sum_p x_pl[p,m] * w1[pl][p, pl'*128+j]
    w1 = np.zeros((2, 128, 256), dtype=np.float64)
    w1[0, :, :128] = U1.real.T
    w1[0, :, 128:] = U1.imag.T
    w1[1, :, :128] = -U1.imag.T
    w1[1, :, 128:] = U1.real.T

    # F2 rhs: col n = j'*256 + pl'*128 + t, t = (q13')(q7'..12')
    perm = np.array([((t & 63) << 1) | (t >> 6) for t in range(128)])
    U2v = U2.reshape(2, 128, 2, 128)[:, perm, :, :]  # [j', t, k, p]
    w2 = np.zeros((2, 2, 128, 512), dtype=np.float64)
    for k in range(2):
        M = U2v[:, :, k, :]  # [j', t, p]
        for jp in range(2):
            w2[0, k, :, jp * 256:jp * 256 + 128] = M[jp].real.T
            w2[0, k, :, jp * 256 + 128:jp * 256 + 256] = M[jp].imag.T
            w2[1, k, :, jp * 256:jp * 256 + 128] = -M[jp].imag.T
            w2[1, k, :, jp * 256 + 128:jp * 256 + 256] = M[jp].real.T

    # P3 lhsT blocks: U3 index (q13, q14..20); block (j=q13', k=q13)
    w3 = np.stack([
        np.stack([_pack_lhsT(U3[j * 128:(j + 1) * 128, k * 128:(k + 1) * 128])
                  for k in (0, 1)])
        for j in (0, 1)])

    return {
        "w1": w1.astype(BF),
        "w2": w2.astype(BF),
        "w3": w3,
        "w4": _pack_lhsT(U4),
    }


# ---------------------------------------------------------------------------
# Bass kernel builder
# ---------------------------------------------------------------------------

def build_nc():
    nc = bacc.Bacc()

    st = nc.declare_dram_parameter("state", [2, 1 << 21], BF16, isOutput=False)
    w1 = nc.declare_dram_parameter("w1", [2, 128, 256], BF16, isOutput=False)
    w2 = nc.declare_dram_parameter("w2", [2, 2, 128, 512], BF16, isOutput=False)
    w3 = nc.declare_dram_parameter("w3", [2, 2, 3, 128, 128], BF16, isOutput=False)
    w4 = nc.declare_dram_parameter("w4", [3, 128, 128], BF16, isOutput=False)
    out = nc.declare_dram_parameter("out", [2, 1 << 21], BF16, isOutput=True)

    # AllToAll bounce buffers, one per quarter v = (q3',q4').
    # Block for dest core b3: [128 rows = q14'..20',
    #                          1024 = (pl,j=q13',e=q5'6',r)]
    a2a_in = [nc.dram_tensor(f"a2a_in{v}", [8, 128, 1024], BF16)
              for v in range(4)]
    a2a_out = [nc.dram_tensor(f"a2a_out{v}", [8, 128, 1024], BF16)
               for v in range(4)]

    with tile.TileContext(nc, num_cores=N_CORES) as tc:
        with tc.tile_pool(name="state", bufs=1) as sp, \
             tc.tile_pool(name="wpool", bufs=1) as wp, \
             tc.tile_pool(name="mm", bufs=4, space="PSUM") as mmp, \
             tc.tile_pool(name="outp", bufs=2) as op:

            A = [sp.tile([128, 16384], BF16, tag=f"A{pl}", name=f"A{pl}")
                 for pl in (0, 1)]
            B = [sp.tile([128, 16384], BF16, tag=f"B{pl}", name=f"B{pl}")
                 for pl in (0, 1)]
            # stg has its own space (deferred F2 chunks still read B);
            # R aliases B[1] (readback is emitted after all F2 reads)
            stg = sp.tile([128, 16384], BF16, tag="stg", name="stg")
            Rt = B[1]

            # ---- weights (gpsimd queue so state load owns sync/scalar) ----
            w1t = []
            for pl in (0, 1):
                t = wp.tile([128, 256], BF16, tag=f"w1_{pl}")
                nc.gpsimd.dma_start(out=t[:], in_=w1[pl])
                w1t.append(t)
            w2t = [[None, None], [None, None]]
            for pl in (0, 1):
                for k in (0, 1):
                    t = wp.tile([128, 512], BF16, tag=f"w2_{pl}{k}")
                    nc.gpsimd.dma_start(out=t[:], in_=w2[pl, k])
                    w2t[pl][k] = t

            def load_w3(dram_ap3, name):
                ts = []
                for i in range(3):
                    t = wp.tile([128, 128], BF16, tag=f"{name}_{i}")
                    nc.gpsimd.dma_start(out=t[:], in_=dram_ap3[i])
                    ts.append(t)
                return ts

            w3t = [[load_w3(w3[j, k], f"w3_{j}{k}") for k in (0, 1)] for j in (0, 1)]
            w4t = load_w3(w4, "w4")

            # ---- state load: partitions (q0..q6), free (q7..q13)x(q14..q20)
            st_v = [st[pl].rearrange("(p f) -> p f", p=128) for pl in (0, 1)]
            for c in range(8):
                for pl in (0, 1):
                    eng = nc.sync if (c + pl) % 2 == 0 else nc.scalar
                    eng.dma_start(
                        out=A[pl][:, c * 2048:(c + 1) * 2048],
                        in_=st_v[pl][:, c * 2048:(c + 1) * 2048])

            # copy engines, rotated per call (only DVE/Act can read PSUM)
            cp_engines = [nc.vector.tensor_copy,
                          lambda out, in_: nc.scalar.copy(out=out, in_=in_)]
            cp_i = [0]

            def copy(out_ap, in_ap):
                cp_engines[cp_i[0] % 2](out_ap, in_ap)
                cp_i[0] += 1

            # ---- F1: U1 fused with transpose (q0..6) -> (q7..13) ----
            # Host pre-permutes shard to free = (b=q14..20)(a=q7..13), so
            # lhsT = A[pl][:, b*128:+128] is contiguous (FWL) and chunk b
            # only needs load-block b//16. psum quad [128,1024] = 4 chunks.
            def f1_quad(b0):
                ps = mmp.tile([128, 1024], F32, tag="mm", name=f"f1_{b0}")
                for i in range(4):
                    b = b0 + i
                    nc.tensor.matmul(ps[:, i * 256:i * 256 + 256],
                                     A[0][:, b * 128:(b + 1) * 128],
                                     w1t[0][:], start=True, stop=False)
                    nc.tensor.matmul(ps[:, i * 256:i * 256 + 256],
                                     A[1][:, b * 128:(b + 1) * 128],
                                     w1t[1][:], start=False, stop=True)
                ps_v = ps[:].rearrange("p (i pl j) -> p i pl j", i=4, pl=2)
                for pl in (0, 1):
                    copy(B[pl][:, b0 * 128:b0 * 128 + 512]
                         .rearrange("p (i j) -> p i j", i=4),
                         ps_v[:, :, pl, :])

            for b0 in range(0, 128, 4):
                f1_quad(b0)

            # ---- F2: U2 fused with transpose (q7..13) -> (q14..20) ----
            # lhsT = B chunk [p=q7..13, m=b=q14..20] for fixed j=(c,k);
            # psum_c[b, (j' 2)(pl' 2)(t 128)] -> A[pl'][b, (c*2+j')*128 + t]
            B_v = [B[pl][:].rearrange("p (b j) -> p j b", j=128) for pl in (0, 1)]

            def f2_pair(c0):
                # two adjacent c-chunks share one [128,1024] psum (2 banks)
                ps = mmp.tile([128, 1024], F32, tag="mm", name=f"f2_{c0}")
                for i in (0, 1):
                    c = c0 + i
                    first = True
                    for k in (0, 1):
                        for pl in (0, 1):
                            nc.tensor.matmul(ps[:, i * 512:i * 512 + 512],
                                             B_v[pl][:, c * 2 + k, :],
                                             w2t[pl][k][:], start=first,
                                             stop=(k == 1 and pl == 1))
                            first = False
                ps_v = ps[:].rearrange("p (i j pl t) -> p i j pl t",
                                       i=2, j=2, pl=2)
                for pl in (0, 1):
                    copy(A[pl][:, c0 * 256:c0 * 256 + 512]
                         .rearrange("p (i j t) -> p i j t", i=2, j=2),
                         ps_v[:, :, :, pl, :])

            # F2 pairs are emitted interleaved with P3 quarters below:
            # quarter (q3,q4) needs only pairs c0 = b3*8 + q3*4 + q4*2.

            # ---- P3: U3 (2x2 blocks over k=q13'); partitions (q14..q20) ----
            # C = A: free = (g=(c,j') 128)(k=q13' 2)(r=q7'..12' 64)
            C_v = [A[pl][:].rearrange("p (g k r) -> p g k r", g=128, k=2)
                   for pl in (0, 1)]
            # stg (=B[0]): free = (b3 8)(q4 2)(pl 2)(j 2)(e 4)(r 64)
            stg_v = stg[:].rearrange("p (b3 q4 pl j e r) -> p b3 q4 pl j e r",
                                     b3=8, q4=2, pl=2, j=2, e=4)

            def p3_piece(b3, q3, q4):
                g0 = b3 * 16 + q3 * 8 + q4 * 4
                xs = {(pl, k): C_v[pl][:, g0:g0 + 4, k, :] for pl in (0, 1)
                      for k in (0, 1)}
                # one [128,1024] psum: (pl 2)(j 2)(e 4)(r 64)
                ps = mmp.tile([128, 1024], F32, tag="mm",
                              name=f"p3_{b3}_{q3}_{q4}")
                pj = [[ps[:, pl * 512 + j * 256:pl * 512 + (j + 1) * 256]
                       for j in (0, 1)] for pl in (0, 1)]
                # start=True clears has_written for the WHOLE bank, so each
                # psum slice's accumulation group must run uninterleaved
                # w.r.t. the other slice sharing its bank.
                for pl in (0, 1):
                    for j in (0, 1):
                        for k in (0, 1):
                            A3, B3, Bn3 = w3t[j][k]
                            if pl == 0:
                                nc.tensor.matmul(pj[0][j], A3[:], xs[(0, k)],
                                                 start=(k == 0), stop=False)
                                nc.tensor.matmul(pj[0][j], Bn3[:], xs[(1, k)],
                                                 start=False, stop=(k == 1))
                            else:
                                nc.tensor.matmul(pj[1][j], A3[:], xs[(1, k)],
                                                 start=(k == 0), stop=False)
                                nc.tensor.matmul(pj[1][j], B3[:], xs[(0, k)],
                                                 start=False, stop=(k == 1))
                copy(stg_v[:, b3, q4], ps[:].rearrange(
                    "p (pl j e r) -> p pl j e r", pl=2, j=2, e=4))

            def stage_dma(b3, q3, q4):
                eng = nc.sync if b3 % 2 == 0 else nc.scalar
                eng.dma_start(out=a2a_in[q3 * 2 + q4][b3],
                              in_=stg_v[:, b3, q4].rearrange(
                                  "p pl j e r -> p (pl j e r)"))

            def collective(v):
                nc.gpsimd.collective_compute(
                    "AllToAll",
                    mybir.AluOpType.bypass,
                    replica_groups=[list(range(N_CORES))],
                    ins=[a2a_in[v].ap().opt()],
                    outs=[a2a_out[v].ap().opt()],
                )

            # ---- tail: readback + P4 + store, per quarter v ----
            # R (=B[1] halves): free = (pl 2)(w3 8)(c 512=(j,e,r))
            ov = [out[pl].rearrange("(p f) -> p f", p=128) for pl in (0, 1)]

            def readback(v):
                # sync queue only — P4 out-DMAs own the scalar queue, so a
                # blocked readback never stalls them (and vice versa)
                R = Rt[:, (v % 2) * 8192:(v % 2) * 8192 + 8192]
                for h3 in range(8):
                    eng = nc.sync
                    eng.dma_start(
                        out=R[h3 * 16:(h3 + 1) * 16, :]
                            .rearrange("m (pl w3 c) -> m pl w3 c", pl=2, w3=8),
                        in_=a2a_out[v][h3].rearrange(
                            "(w3 m) (pl c) -> m pl w3 c", m=16, pl=2))
                return R

            def p4_chunk(v, R, otv, w3c):
                A4, B4, Bn4 = w4t
                rre = R[:, w3c * 512:(w3c + 1) * 512]
                rim = R[:, 4096 + w3c * 512:4096 + (w3c + 1) * 512]
                ps = mmp.tile([128, 1024], F32, tag="mm", name=f"p4_{v}_{w3c}")
                pre = ps[:, 0:512]
                pim = ps[:, 512:1024]
                nc.tensor.matmul(pre, A4[:], rre, start=True, stop=False)
                nc.tensor.matmul(pim, A4[:], rim, start=True, stop=False)
                nc.tensor.matmul(pim, B4[:], rre, start=False, stop=True)
                nc.tensor.matmul(pre, Bn4[:], rim, start=False, stop=True)
                copy(otv[:, w3c * 1024:(w3c + 1) * 1024], ps[:])

            def p4_flush(v, otv):
                # 2 DMAs of 1 MB per quarter instead of 16 small ones
                ot_v = otv[:].rearrange("p (w3 pl c) -> p w3 pl c", w3=8, pl=2)
                for pl in (0, 1):
                    nc.scalar.dma_start(
                        out=ov[pl][:, v * 4096:(v + 1) * 4096],
                        in_=ot_v[:, :, pl, :])

            # ---- emission ----
            for q3 in (0, 1):
                for q4 in (0, 1):
                    for b3 in range(8):
                        f2_pair(b3 * 8 + q3 * 4 + q4 * 2)
                    for b3 in range(8):
                        p3_piece(b3, q3, q4)
                        stage_dma(b3, q3, q4)
                    collective(q3 * 2 + q4)
            for v in range(4):
                R = readback(v)
                otv = op.tile([128, 8192], BF16, tag="p4out",
                              name=f"p4out_{v}")
                for w3c in range(8):
                    p4_chunk(v, R, otv, w3c)
                p4_flush(v, otv)

    return nc


# ---------------------------------------------------------------------------
# Host wrapper
# ---------------------------------------------------------------------------

TRACE = False
LAST_EXEC_NS = None
LAST_RESULTS = None


def kernel(state, gates1, gates2):
    global LAST_EXEC_NS, LAST_RESULTS
    state = np.asarray(state, dtype=np.float32)
    weights = build_weights(np.asarray(gates1, dtype=np.float32),
                            np.asarray(gates2, dtype=np.float32))

    # shard over (q21,q22,q23) = index mod 8, cast to bf16.
    # Local free layout is (q14..20)(q7..13) — a/b swapped so F1's lhsT
    # chunks are contiguous in SBUF.
    shards = state.reshape(2, 1 << 21, 8).transpose(2, 0, 1)
    shards = np.ascontiguousarray(
        shards.reshape(8, 2, 128, 128, 128).swapaxes(3, 4)
    ).reshape(8, 2, 1 << 21).astype(BF)

    nc = build_nc()
    if not nc.is_finalized():
        nc.finalize()
    in_maps = [dict(weights, state=shards[d]) for d in range(N_CORES)]
    res = run_bass_kernel_spmd(nc, in_maps, core_ids=list(range(N_CORES)),
                               trace=TRACE)
    LAST_EXEC_NS = res.exec_time_ns
    LAST_RESULTS = res

    return unshard([res.results[d]["out"] for d in range(N_CORES)])


def unshard(outs):
    # core d holds (q0,q1,q2) = d;
    # local layout: [pl][p = (q21,q22,q23,q17,q18,q19,q20)]
    #               [f = (q3,q4)(q14,q15,q16)(q13)(q5,q6)(q7..q12)]
    full = np.empty((2, 8) + (2,) * 21, dtype=np.float32)
    bits = [21, 22, 23, 17, 18, 19, 20, 3, 4, 14, 15, 16, 13, 5, 6,
            7, 8, 9, 10, 11, 12]
    perm = [bits.index(3 + i) for i in range(21)]
    for d in range(N_CORES):
        od = np.asarray(outs[d]).astype(np.float32).reshape((2,) + (2,) * 21)
        full[:, d] = np.transpose(od, [0] + [1 + p for p in perm])
    return full.reshape(2, 1 << 24)


if __name__ == "__main__":
    rng = np.random.default_rng(0)
    state = rng.standard_normal((2, 1 << 24)).astype(np.float32)
    g1 = rng.standard_normal((24, 2, 2, 2)).astype(np.float32)
    g2 = rng.standard_normal((23, 2, 4, 4)).astype(np.float32)
    out = kernel(state, g1, g2)
    print(out.shape, out.dtype)
